# revision 12
# baseline (speedup 1.0000x reference)
"""Trainium2 Bass kernel for nn_Decoder_GRU (Chebyshev graph-conv GRU decoder).

Strategy (8 NeuronCores, SPMD):
- Row-shard both Laplacians: edge 512 rows/core (L^T shard SBUF-resident, 8MB),
  node 128 rows/core.
- Algebra: feature transforms commute with graph propagation, so only the
  F-wide hidden state is propagated (Yh = L@hx, Zh = L@Yh); the 2F-wide concat
  never materializes. Input-dependent terms (Ei = L@X, Fi = L@Ei) are computed
  once in a setup phase and folded into each step's gate matmuls.
- Each Cheb hop contracts over the full graph dim, so the hop input is
  AllGathered across cores (fused edge+node buffers, 4 gathers/step).
- Layouts: hop outputs land "B-layout" [b*F+f, mr] (batch folded on the
  partition axis); block-diagonal weights make all feature matmuls single
  [128,128,mr] matmuls. PE-transposes produce the "A-layout" [m, b*F+f]
  shards the next hop consumes as its stationary operand.
- All matmuls run as float32r (full-rate fp32 path), typed end-to-end.

kernel(**inputs) takes the FULL inputs and returns (out_node, out_edge)
matching reference.reference().
"""
import sys
if '/opt/trn_rl_repo' not in sys.path:
    sys.path.insert(0, '/opt/trn_rl_repo')

import numpy as np
import concourse.bacc as bacc
import concourse.mybir as mybir
import concourse.tile as tile
from concourse import bass_utils

NC = 8          # cores
B = 4           # batch
F = 32          # features
BF = B * F      # 128
FP = mybir.dt.float32
FR = mybir.dt.float32r
AFT = mybir.ActivationFunctionType

# graph geometry: (M, rows-per-core, k-chunks, A-blocks-per-shard)
GEO = {
    'e': dict(M=4096, MR=512, KC=32, NB=4),
    'n': dict(M=1024, MR=128, KC=8, NB=1),
}
AG_ROWS = GEO['e']['MR'] + GEO['n']['MR']   # 640 rows per rank in gather bufs
NODE_OFF = GEO['e']['MR']                   # node rows start at 512

# weight-tile indices in W_all [128, 19, 128]
W_RS_X, W_RS_EI, W_RS_FI = 0, 1, 2          # r statics
W_US_X, W_US_EI, W_US_FI = 3, 4, 5          # u statics
W_CS_X, W_CS_EI, W_CS_FI = 6, 7, 8          # c statics
W_R_HX, W_R_YH, W_R_ZH = 9, 10, 11          # r dynamics
W_U_HX, W_U_YH, W_U_ZH = 12, 13, 14         # u dynamics
W_C_RH, W_C_YR, W_C_ZR = 15, 16, 17         # c dynamics
W_Y = 18                                    # output proj


def _blockdiag(w):
    """w: (F, G) -> block-diag over B batches: (B*F, B*G)."""
    Fi, G = w.shape
    out = np.zeros((B * Fi, B * G), np.float32)
    for b in range(B):
        out[b * Fi:(b + 1) * Fi, b * G:(b + 1) * G] = w
    return out


def _weights_for_graph(Wg, Wu, Wy):
    """Build W_all (128, 19, 128) from Wg (3,2F,2F), Wu (3,2F,F), Wy (F,F)."""
    Wg_sum = Wg[0] - Wg[2]
    Wu_sum = Wu[0] - Wu[2]
    g_t = [Wg_sum[:F], Wg[1][:F], 2.0 * Wg[2][:F]]      # (F, 2F) tops
    g_b = [Wg_sum[F:], Wg[1][F:], 2.0 * Wg[2][F:]]      # (F, 2F) bottoms
    u_t = [Wu_sum[:F], Wu[1][:F], 2.0 * Wu[2][:F]]      # (F, F) tops
    u_b = [Wu_sum[F:], Wu[1][F:], 2.0 * Wu[2][F:]]      # (F, F) bottoms
    slots = [None] * 19
    for k in range(3):
        slots[W_RS_X + k] = g_t[k][:, :F]
        slots[W_US_X + k] = g_t[k][:, F:]
        slots[W_CS_X + k] = u_t[k]
        slots[W_R_HX + k] = g_b[k][:, :F]
        slots[W_U_HX + k] = g_b[k][:, F:]
        slots[W_C_RH + k] = u_b[k]
    slots[W_Y] = Wy
    return np.stack([_blockdiag(np.asarray(s, np.float32)) for s in slots], 1)


def _a_layout(x):
    """(B, M, F) -> A-layout (M, B*F)."""
    return np.ascontiguousarray(np.transpose(x, (1, 0, 2)).reshape(x.shape[1], BF))


def _b_layout_shard(x, r, mr):
    """(B, M, F) -> B-layout shard (B*F, mr) for core r."""
    sl = x[:, r * mr:(r + 1) * mr, :]           # (B, mr, F)
    return np.ascontiguousarray(np.transpose(sl, (0, 2, 1)).reshape(BF, mr))


def _build(T):
    nc = bacc.Bacc("TRN2", target_bir_lowering=False, debug=False, num_devices=NC)

    # ---- DRAM I/O ----
    din = {}
    for g in ('e', 'n'):
        M, MR = GEO[g]['M'], GEO[g]['MR']
        din[f'LT_{g}'] = nc.dram_tensor(f"LT_{g}", [M, MR], FR, kind="ExternalInput")
        din[f'XA_{g}'] = nc.dram_tensor(f"XA_{g}", [M, BF], FR, kind="ExternalInput")
        din[f'XB_{g}'] = nc.dram_tensor(f"XB_{g}", [BF, MR], FR, kind="ExternalInput")
        din[f'W_{g}'] = nc.dram_tensor(f"W_{g}", [BF, 19, BF], FR, kind="ExternalInput")
        din[f'bias_{g}'] = nc.dram_tensor(f"bias_{g}", [BF, 1], FP, kind="ExternalInput")
    din['ident'] = nc.dram_tensor("ident", [128, 128], FR, kind="ExternalInput")
    dout = {
        'e': nc.dram_tensor("out_e", [T, BF, GEO['e']['MR']], FR, kind="ExternalOutput"),
        'n': nc.dram_tensor("out_n", [T, BF, GEO['n']['MR']], FR, kind="ExternalOutput"),
    }

    with tile.TileContext(nc) as tc:
        with (
            tc.tile_pool(name="res", bufs=1) as res,            # resident tiles
            tc.tile_pool(name="st", bufs=2) as st,              # per-step state tiles
            tc.tile_pool(name="lhsT_e", bufs=10) as lhsT_e,     # hop stationary stream
            tc.tile_pool(name="lhsT_n", bufs=4) as lhsT_n,
            tc.tile_pool(name="ps_hop", bufs=2, space="PSUM") as ps_hop,
            tc.tile_pool(name="ps_tr", bufs=2, space="PSUM") as ps_tr,
            tc.tile_pool(name="ps_sm", bufs=2, space="PSUM") as ps_sm,
            tc.tile_pool(name="dram", bufs=1, space="DRAM") as dram,
        ):
            # ---- resident loads ----
            lt = {}
            for g in ('e', 'n'):
                M, MR, KC = GEO[g]['M'], GEO[g]['MR'], GEO[g]['KC']
                tiles = []
                ngrp = KC // 8
                for gi in range(ngrp):
                    t = res.tile([128, 8, MR], FR, name=f"lt_{g}{gi}")
                    nc.sync.dma_start(
                        t[:], din[f'LT_{g}'].ap()[gi * 1024:(gi + 1) * 1024, :]
                        .rearrange("(c p) m -> p c m", p=128))
                    tiles.append(t)
                lt[g] = tiles

            xb, wt, bias = {}, {}, {}
            for g in ('e', 'n'):
                MR = GEO[g]['MR']
                xb[g] = res.tile([BF, MR], FR, name=f"xb_{g}")
                nc.sync.dma_start(xb[g][:], din[f'XB_{g}'][:])
                wt[g] = res.tile([BF, 19, BF], FR, name=f"wt_{g}")
                nc.sync.dma_start(wt[g][:], din[f'W_{g}'][:])
                bias[g] = res.tile([BF, 1], FP, name=f"bias_{g}")
                nc.sync.dma_start(bias[g][:], din[f'bias_{g}'][:])
            ident = res.tile([128, 128], FR, name="ident")
            nc.sync.dma_start(ident[:], din['ident'][:])

            def ltc(g, k):
                return lt[g][k // 8][:, k % 8, :]

            # ---- helpers ----
            def hop(g, src_dram, gathered, tag):
                """Yout.T[bf, mr] = sum_m src[m, bf] * LT[m, mr].
                `gathered`: src is an AllGather output [NC*640, BF] (this graph's
                rows inside each rank region); else src is a contiguous [M, BF].
                Returns the psum tile (caller copies/consumes)."""
                M, MR, KC, NB = (GEO[g][k] for k in ('M', 'MR', 'KC', 'NB'))
                psum = ps_hop.tile([BF, MR], FP, name=f"hops_{g}", tag="hop")
                chunks = []
                if g == 'e':
                    for rr in range(NC):
                        tl = lhsT_e.tile([128, NB, BF], FR,
                                         name=f"lh_{tag}{rr}", tag="lhsT_e")
                        base = rr * AG_ROWS if gathered else rr * MR
                        nc.sync.dma_start(
                            tl[:], src_dram[base:base + MR, :]
                            .rearrange("(j p) f -> p j f", p=128))
                        for j in range(NB):
                            chunks.append(tl[:, j, :])
                else:
                    for half in range(2):
                        tl = lhsT_n.tile([128, 4, BF], FR,
                                         name=f"lh_{tag}h{half}", tag="lhsT_n")
                        rr0 = half * 4
                        if gathered:
                            src = (src_dram[0:NC * AG_ROWS, :]
                                   .rearrange("(rr x) f -> rr x f", rr=NC)
                                   [rr0:rr0 + 4, NODE_OFF:NODE_OFF + 128, :]
                                   .rearrange("rr p f -> p rr f"))
                        else:
                            src = (src_dram[rr0 * 128:(rr0 + 4) * 128, :]
                                   .rearrange("(c p) f -> p c f", p=128))
                        nc.sync.dma_start(tl[:], src)
                        for j in range(4):
                            chunks.append(tl[:, j, :])
                for k in range(KC):
                    nc.tensor.matmul(psum[:], chunks[k], ltc(g, k),
                                     start=(k == 0), stop=(k == KC - 1))
                return psum

            def hop_to_sbuf(g, src_dram, gathered, tag, pool=None):
                psum = hop(g, src_dram, gathered, tag)
                out = (pool or st).tile([BF, GEO[g]['MR']], FR,
                                        name=f"{tag}_{g}", tag=f"{tag}_{g}")
                nc.vector.tensor_copy(out[:], psum[:])
                return out

            def transpose_into(g, bt, ag_in):
                """PE-transpose B-tile -> A-layout rows of ag_in (this rank's region)."""
                NB, MR = GEO[g]['NB'], GEO[g]['MR']
                tp = ps_tr.tile([128, MR], FR, name=f"tp_{g}", tag="tp")
                for j in range(NB):
                    nc.tensor.transpose(tp[:, j * 128:(j + 1) * 128],
                                        bt[:, j * 128:(j + 1) * 128], ident[:])
                stage = st.tile([128, MR], FR, name=f"tps_{g}", tag=f"tps_{g}")
                nc.vector.tensor_copy(stage[:], tp[:])
                off = 0 if g == 'e' else NODE_OFF
                nc.sync.dma_start(
                    ag_in[off:off + MR, :].rearrange("(j m) f -> m j f", m=128),
                    stage[:].rearrange("m (j f) -> m j f", f=BF))

            ag_count = [0]

            def allgather(write_fn):
                """write_fn(ag_in) fills this rank's [640,128] shard; returns gathered
                [8*640, 128] dram tensor."""
                i = ag_count[0]
                ag_count[0] += 1
                ag_in = dram.tile([AG_ROWS, BF], FR, name=f"agi{i}")
                ag_out, _ = tc.tile([NC * AG_ROWS, BF], FR, space="DRAM",
                                    addr_space="Shared", name=f"ago{i}")
                write_fn(ag_in)
                nc.gpsimd.collective_compute(
                    "AllGather", mybir.AluOpType.bypass,
                    replica_groups=[list(range(NC))],
                    ins=[ag_in[:].opt()], outs=[ag_out[:].opt()],
                )
                return ag_out

            def smalls(g, idxs, rhss, act, out_name, act_bias=None):
                """psum = sum_i W[idxs[i]].T @ rhss[i]; out = act(psum [+bias])."""
                MR = GEO[g]['MR']
                psum = ps_sm.tile([BF, MR], FP, name=f"sm_{g}", tag="sm")
                n = len(idxs)
                for i, (ix, rh) in enumerate(zip(idxs, rhss)):
                    nc.tensor.matmul(psum[:], wt[g][:, ix, :], rh[:],
                                     start=(i == 0), stop=(i == n - 1))
                out = st.tile([BF, MR], FR, name=f"{out_name}_{g}", tag=f"{out_name}_{g}")
                if act_bias is not None:
                    nc.scalar.activation(out[:], psum[:], act, bias=act_bias)
                else:
                    nc.scalar.activation(out[:], psum[:], act)
                return out

            # ---- setup: Ei = L@X, Fi = L@Ei ----
            eiB, fiB = {}, {}
            for g in ('e', 'n'):
                eiB[g] = hop_to_sbuf(g, din[f'XA_{g}'], False, "Ei", pool=res)

            g_ei = allgather(lambda agi: [transpose_into(g, eiB[g], agi)
                                          for g in ('e', 'n')])
            for g in ('e', 'n'):
                fiB[g] = hop_to_sbuf(g, g_ei, True, "Fi", pool=res)

            # ---- step 0 (hx = 0) ----
            hB, out_tiles = {}, {}
            for g in ('e', 'n'):
                stat = [xb[g], eiB[g], fiB[g]]
                r0 = smalls(g, [W_RS_X, W_RS_EI, W_RS_FI], stat, AFT.Sigmoid, "r")
                u0 = smalls(g, [W_US_X, W_US_EI, W_US_FI], stat, AFT.Sigmoid, "u")
                c0 = smalls(g, [W_CS_X, W_CS_EI, W_CS_FI], stat, AFT.Tanh, "c")
                MR = GEO[g]['MR']
                tmp = st.tile([BF, MR], FR, name=f"tmp_{g}", tag=f"tmp_{g}")
                nc.vector.tensor_mul(tmp[:], u0[:], c0[:])
                hy = st.tile([BF, MR], FR, name=f"hy_{g}", tag=f"hy_{g}")
                nc.vector.tensor_sub(hy[:], c0[:], tmp[:])
                hB[g] = hy
                y0 = smalls(g, [W_Y], [hy], AFT.Sigmoid, "y", act_bias=bias[g][:, 0:1])
                nc.sync.dma_start(dout[g].ap()[0], y0[:])

            if T > 1:
                g_hy = allgather(lambda agi: [transpose_into(g, hB[g], agi)
                                              for g in ('e', 'n')])

            # ---- steps 1..T-1 ----
            for t in range(1, T):
                yhB = {g: hop_to_sbuf(g, g_hy, True, "Yh")
                       for g in ('e', 'n')}
                g_yh = allgather(lambda agi: [transpose_into(g, yhB[g], agi)
                                              for g in ('e', 'n')])
                zhB = {g: hop_to_sbuf(g, g_yh, True, "Zh")
                       for g in ('e', 'n')}

                rhB = {}
                for g in ('e', 'n'):
                    stat = [xb[g], eiB[g], fiB[g], hB[g], yhB[g], zhB[g]]
                    r = smalls(g, [W_RS_X, W_RS_EI, W_RS_FI, W_R_HX, W_R_YH, W_R_ZH],
                               stat, AFT.Sigmoid, "r")
                    u = smalls(g, [W_US_X, W_US_EI, W_US_FI, W_U_HX, W_U_YH, W_U_ZH],
                               stat, AFT.Sigmoid, "u")
                    MR = GEO[g]['MR']
                    rh = st.tile([BF, MR], FR, name=f"rh_{g}", tag=f"rh_{g}")
                    nc.vector.tensor_mul(rh[:], r[:], hB[g][:])
                    rhB[g] = rh
                    # stash u for the hy update
                    out_tiles[f'u_{g}'] = u

                g_rh = allgather(lambda agi: [transpose_into(g, rhB[g], agi)
                                              for g in ('e', 'n')])
                yrB = {g: hop_to_sbuf(g, g_rh, True, "Yr")
                       for g in ('e', 'n')}
                g_yr = allgather(lambda agi: [transpose_into(g, yrB[g], agi)
                                              for g in ('e', 'n')])
                zrB = {g: hop_to_sbuf(g, g_yr, True, "Zr")
                       for g in ('e', 'n')}

                for g in ('e', 'n'):
                    c = smalls(g, [W_CS_X, W_CS_EI, W_CS_FI, W_C_RH, W_C_YR, W_C_ZR],
                               [xb[g], eiB[g], fiB[g], rhB[g], yrB[g], zrB[g]],
                               AFT.Tanh, "c")
                    u = out_tiles[f'u_{g}']
                    MR = GEO[g]['MR']
                    tmp = st.tile([BF, MR], FR, name=f"tmp_{g}", tag=f"tmp_{g}")
                    nc.vector.tensor_sub(tmp[:], hB[g][:], c[:])
                    tmp2 = st.tile([BF, MR], FR, name=f"tmp2_{g}", tag=f"tmp2_{g}")
                    nc.vector.tensor_mul(tmp2[:], u[:], tmp[:])
                    hy = st.tile([BF, MR], FR, name=f"hy_{g}", tag=f"hy_{g}")
                    nc.vector.tensor_add(hy[:], c[:], tmp2[:])
                    hB[g] = hy
                    y = smalls(g, [W_Y], [hy], AFT.Sigmoid, "y",
                               act_bias=bias[g][:, 0:1])
                    nc.sync.dma_start(dout[g].ap()[t], y[:])

                if t < T - 1:
                    g_hy = allgather(lambda agi: [transpose_into(g, hB[g], agi)
                                                  for g in ('e', 'n')])

    nc.compile()
    return nc


_CACHE = {}


def _get_nc(T):
    if T not in _CACHE:
        _CACHE[T] = _build(T)
    return _CACHE[T]


def _in_maps(ins):
    return _prepare(**{k: ins[k] for k in (
        'inputs_node', 'inputs_edge', 'L_node', 'L_edge', 'Wg_node', 'Wg_edge',
        'Wu_node', 'Wu_edge', 'W_node', 'b_node', 'W_edge', 'b_edge')})


def _prepare(inputs_node, inputs_edge, L_node, L_edge, Wg_node, Wg_edge,
             Wu_node, Wu_edge, W_node, b_node, W_edge, b_edge):
    inputs_node = np.asarray(inputs_node, np.float32)
    inputs_edge = np.asarray(inputs_edge, np.float32)
    L_node = np.asarray(L_node, np.float32)
    L_edge = np.asarray(L_edge, np.float32)

    W_all = {
        'n': _weights_for_graph(np.asarray(Wg_node, np.float32),
                                np.asarray(Wu_node, np.float32),
                                np.asarray(W_node, np.float32)),
        'e': _weights_for_graph(np.asarray(Wg_edge, np.float32),
                                np.asarray(Wu_edge, np.float32),
                                np.asarray(W_edge, np.float32)),
    }
    XA = {'n': _a_layout(inputs_node), 'e': _a_layout(inputs_edge)}
    L_ = {'n': L_node, 'e': L_edge}
    X_ = {'n': inputs_node, 'e': inputs_edge}
    bias_ = {'n': np.asarray(b_node, np.float32), 'e': np.asarray(b_edge, np.float32)}
    ident = np.eye(128, dtype=np.float32)

    in_maps = []
    for r in range(NC):
        m = {'ident': ident}
        for g in ('e', 'n'):
            MR = GEO[g]['MR']
            m[f'LT_{g}'] = np.ascontiguousarray(L_[g][r * MR:(r + 1) * MR, :].T)
            m[f'XA_{g}'] = XA[g]
            m[f'XB_{g}'] = _b_layout_shard(X_[g], r, MR)
            m[f'W_{g}'] = W_all[g]
            m[f'bias_{g}'] = np.tile(bias_[g], B)[:, None]
        in_maps.append(m)
    return in_maps


def kernel(inputs_node, inputs_edge, L_node, L_edge, Wg_node, Wg_edge,
           Wu_node, Wu_edge, W_node, b_node, W_edge, b_edge, seq_target):
    T = int(seq_target)
    in_maps = _prepare(inputs_node, inputs_edge, L_node, L_edge, Wg_node,
                       Wg_edge, Wu_node, Wu_edge, W_node, b_node, W_edge, b_edge)
    nc = _get_nc(T)
    res = bass_utils.run_bass_kernel_spmd(nc, in_maps, core_ids=list(range(NC)))

    outs = {}
    for g, Mfull in (('n', GEO['n']['M']), ('e', GEO['e']['M'])):
        MR = GEO[g]['MR']
        full = np.empty((T, B, Mfull, F), np.float32)
        for r in range(NC):
            y = res.results[r][f'out_{g}']          # (T, BF, MR)
            y = y.reshape(T, B, F, MR).transpose(0, 1, 3, 2)
            full[:, :, r * MR:(r + 1) * MR, :] = y
        outs[g] = full
    return outs['n'], outs['e']


def timed_run(ins):
    """Re-run with NTFF tracing; returns max-core exec time in ns (or None)."""
    T = int(ins['seq_target'])
    nc = _get_nc(T)
    in_maps = _in_maps(ins)
    res = bass_utils.run_bass_kernel_spmd(nc, in_maps, core_ids=list(range(NC)),
                                          trace=True)
    return res.exec_time_ns


if __name__ == "__main__":
    # smoke test with random inputs matching spec shapes
    rng = np.random.default_rng(0)
    ins = dict(
        inputs_node=rng.standard_normal((B, 1024, F), dtype=np.float32),
        inputs_edge=rng.standard_normal((B, 4096, F), dtype=np.float32),
        L_node=(rng.standard_normal((1024, 1024)) / 32).astype(np.float32),
        L_edge=(rng.standard_normal((4096, 4096)) / 64).astype(np.float32),
        Wg_node=(rng.standard_normal((3, 64, 64)) / 8).astype(np.float32),
        Wg_edge=(rng.standard_normal((3, 64, 64)) / 8).astype(np.float32),
        Wu_node=(rng.standard_normal((3, 64, 32)) / 8).astype(np.float32),
        Wu_edge=(rng.standard_normal((3, 64, 32)) / 8).astype(np.float32),
        W_node=(rng.standard_normal((32, 32)) / 5.6).astype(np.float32),
        b_node=np.zeros(32, np.float32),
        W_edge=(rng.standard_normal((32, 32)) / 5.6).astype(np.float32),
        b_edge=np.zeros(32, np.float32),
        seq_target=6,
    )
    on, oe = kernel(**ins)
    print("node out", on.shape, "edge out", oe.shape)


# revision 14
# speedup vs baseline: 1.0043x; 1.0043x over previous
"""Trainium2 Bass kernel for nn_Decoder_GRU (Chebyshev graph-conv GRU decoder).

Strategy (8 NeuronCores, SPMD):
- Row-shard both Laplacians: edge 512 rows/core (L^T shard SBUF-resident, 8MB),
  node 128 rows/core.
- Algebra: feature transforms commute with graph propagation, so only the
  F-wide hidden state is propagated (Yh = L@hx, Zh = L@Yh); the 2F-wide concat
  never materializes. Input-dependent terms (Ei = L@X, Fi = L@Ei) are computed
  once in a setup phase and folded into each step's gate matmuls.
- Each Cheb hop contracts over the full graph dim, so the hop input is
  AllGathered across cores (fused edge+node buffers, 4 gathers/step).
- Layouts: hop outputs land "B-layout" [b*F+f, mr] (batch folded on the
  partition axis); block-diagonal weights make all feature matmuls single
  [128,128,mr] matmuls. PE-transposes produce the "A-layout" [m, b*F+f]
  shards the next hop consumes as its stationary operand.
- All matmuls run as float32r (full-rate fp32 path), typed end-to-end.

kernel(**inputs) takes the FULL inputs and returns (out_node, out_edge)
matching reference.reference().
"""
import sys
if '/opt/trn_rl_repo' not in sys.path:
    sys.path.insert(0, '/opt/trn_rl_repo')

import numpy as np
import concourse.bacc as bacc
import concourse.mybir as mybir
import concourse.tile as tile
from concourse import bass_utils

NC = 8          # cores
B = 4           # batch
F = 32          # features
BF = B * F      # 128
FP = mybir.dt.float32
FR = mybir.dt.float32r
AFT = mybir.ActivationFunctionType

# graph geometry: (M, rows-per-core, k-chunks, A-blocks-per-shard)
GEO = {
    'e': dict(M=4096, MR=512, KC=32, NB=4),
    'n': dict(M=1024, MR=128, KC=8, NB=1),
}
AG_ROWS = GEO['e']['MR'] + GEO['n']['MR']   # 640 rows per rank in gather bufs
NODE_OFF = GEO['e']['MR']                   # node rows start at 512

# weight-tile indices in W_all [128, 19, 128]
W_RS_X, W_RS_EI, W_RS_FI = 0, 1, 2          # r statics
W_US_X, W_US_EI, W_US_FI = 3, 4, 5          # u statics
W_CS_X, W_CS_EI, W_CS_FI = 6, 7, 8          # c statics
W_R_HX, W_R_YH, W_R_ZH = 9, 10, 11          # r dynamics
W_U_HX, W_U_YH, W_U_ZH = 12, 13, 14         # u dynamics
W_C_RH, W_C_YR, W_C_ZR = 15, 16, 17         # c dynamics
W_Y = 18                                    # output proj


def _blockdiag(w):
    """w: (F, G) -> block-diag over B batches: (B*F, B*G)."""
    Fi, G = w.shape
    out = np.zeros((B * Fi, B * G), np.float32)
    for b in range(B):
        out[b * Fi:(b + 1) * Fi, b * G:(b + 1) * G] = w
    return out


def _weights_for_graph(Wg, Wu, Wy):
    """Build W_all (128, 19, 128) from Wg (3,2F,2F), Wu (3,2F,F), Wy (F,F)."""
    Wg_sum = Wg[0] - Wg[2]
    Wu_sum = Wu[0] - Wu[2]
    g_t = [Wg_sum[:F], Wg[1][:F], 2.0 * Wg[2][:F]]      # (F, 2F) tops
    g_b = [Wg_sum[F:], Wg[1][F:], 2.0 * Wg[2][F:]]      # (F, 2F) bottoms
    u_t = [Wu_sum[:F], Wu[1][:F], 2.0 * Wu[2][:F]]      # (F, F) tops
    u_b = [Wu_sum[F:], Wu[1][F:], 2.0 * Wu[2][F:]]      # (F, F) bottoms
    slots = [None] * 19
    for k in range(3):
        slots[W_RS_X + k] = g_t[k][:, :F]
        slots[W_US_X + k] = g_t[k][:, F:]
        slots[W_CS_X + k] = u_t[k]
        slots[W_R_HX + k] = g_b[k][:, :F]
        slots[W_U_HX + k] = g_b[k][:, F:]
        slots[W_C_RH + k] = u_b[k]
    slots[W_Y] = Wy
    return np.stack([_blockdiag(np.asarray(s, np.float32)) for s in slots], 1)


def _a_layout(x):
    """(B, M, F) -> A-layout (M, B*F)."""
    return np.ascontiguousarray(np.transpose(x, (1, 0, 2)).reshape(x.shape[1], BF))


def _b_layout_shard(x, r, mr):
    """(B, M, F) -> B-layout shard (B*F, mr) for core r."""
    sl = x[:, r * mr:(r + 1) * mr, :]           # (B, mr, F)
    return np.ascontiguousarray(np.transpose(sl, (0, 2, 1)).reshape(BF, mr))


def _build(T):
    nc = bacc.Bacc("TRN2", target_bir_lowering=False, debug=False, num_devices=NC)

    # ---- DRAM I/O ----
    din = {}
    for g in ('e', 'n'):
        M, MR = GEO[g]['M'], GEO[g]['MR']
        din[f'LT_{g}'] = nc.dram_tensor(f"LT_{g}", [M, MR], FR, kind="ExternalInput")
        din[f'XA_{g}'] = nc.dram_tensor(f"XA_{g}", [M, BF], FR, kind="ExternalInput")
        din[f'XB_{g}'] = nc.dram_tensor(f"XB_{g}", [BF, MR], FR, kind="ExternalInput")
        din[f'W_{g}'] = nc.dram_tensor(f"W_{g}", [BF, 19, BF], FR, kind="ExternalInput")
        din[f'bias_{g}'] = nc.dram_tensor(f"bias_{g}", [BF, 1], FP, kind="ExternalInput")
    din['ident'] = nc.dram_tensor("ident", [128, 128], FR, kind="ExternalInput")
    dout = {
        'e': nc.dram_tensor("out_e", [T, BF, GEO['e']['MR']], FR, kind="ExternalOutput"),
        'n': nc.dram_tensor("out_n", [T, BF, GEO['n']['MR']], FR, kind="ExternalOutput"),
    }

    with tile.TileContext(nc) as tc:
        with (
            tc.tile_pool(name="res", bufs=1) as res,            # resident tiles
            tc.tile_pool(name="st", bufs=2) as st,              # per-step state tiles
            tc.tile_pool(name="lhsT_e", bufs=10) as lhsT_e,     # hop stationary stream
            tc.tile_pool(name="lhsT_n", bufs=4) as lhsT_n,
            tc.tile_pool(name="ps_hop", bufs=2, space="PSUM") as ps_hop,
            tc.tile_pool(name="ps_tr", bufs=2, space="PSUM") as ps_tr,
            tc.tile_pool(name="ps_sm", bufs=2, space="PSUM") as ps_sm,
            tc.tile_pool(name="dram", bufs=1, space="DRAM") as dram,
        ):
            # ---- resident loads ----
            lt = {}
            for g in ('e', 'n'):
                M, MR, KC = GEO[g]['M'], GEO[g]['MR'], GEO[g]['KC']
                tiles = []
                ngrp = KC // 8
                for gi in range(ngrp):
                    t = res.tile([128, 8, MR], FR, name=f"lt_{g}{gi}")
                    nc.sync.dma_start(
                        t[:], din[f'LT_{g}'].ap()[gi * 1024:(gi + 1) * 1024, :]
                        .rearrange("(c p) m -> p c m", p=128))
                    tiles.append(t)
                lt[g] = tiles

            xb, wt, bias = {}, {}, {}
            for g in ('e', 'n'):
                MR = GEO[g]['MR']
                xb[g] = res.tile([BF, MR], FR, name=f"xb_{g}")
                nc.sync.dma_start(xb[g][:], din[f'XB_{g}'][:])
                wt[g] = res.tile([BF, 19, BF], FR, name=f"wt_{g}")
                nc.sync.dma_start(wt[g][:], din[f'W_{g}'][:])
                bias[g] = res.tile([BF, 1], FP, name=f"bias_{g}")
                nc.sync.dma_start(bias[g][:], din[f'bias_{g}'][:])
            ident = res.tile([128, 128], FR, name="ident")
            nc.sync.dma_start(ident[:], din['ident'][:])

            def ltc(g, k):
                return lt[g][k // 8][:, k % 8, :]

            # ---- helpers ----
            def hop(g, src_dram, tag):
                """Yout.T[bf, mr] = sum_m src[m, bf] * LT[m, mr].
                src is a contiguous [M, BF] dram tensor."""
                M, MR, KC, NB = (GEO[g][k] for k in ('M', 'MR', 'KC', 'NB'))
                psum = ps_hop.tile([BF, MR], FP, name=f"hops_{g}", tag="hop")
                chunks = []
                if g == 'e':
                    for rr in range(NC):
                        tl = lhsT_e.tile([128, NB, BF], FR,
                                         name=f"lh_{tag}{rr}", tag="lhsT_e")
                        base = rr * MR
                        nc.sync.dma_start(
                            tl[:], src_dram[base:base + MR, :]
                            .rearrange("(j p) f -> p j f", p=128))
                        for j in range(NB):
                            chunks.append(tl[:, j, :])
                else:
                    for half in range(2):
                        tl = lhsT_n.tile([128, 4, BF], FR,
                                         name=f"lh_{tag}h{half}", tag="lhsT_n")
                        rr0 = half * 4
                        src = (src_dram[rr0 * 128:(rr0 + 4) * 128, :]
                               .rearrange("(c p) f -> p c f", p=128))
                        nc.sync.dma_start(tl[:], src)
                        for j in range(4):
                            chunks.append(tl[:, j, :])
                for k in range(KC):
                    nc.tensor.matmul(psum[:], chunks[k], ltc(g, k),
                                     start=(k == 0), stop=(k == KC - 1))
                return psum

            def hop_to_sbuf(g, src_dram, tag, pool=None):
                psum = hop(g, src_dram, tag)
                out = (pool or st).tile([BF, GEO[g]['MR']], FR,
                                        name=f"{tag}_{g}", tag=f"{tag}_{g}")
                nc.vector.tensor_copy(out[:], psum[:])
                return out

            def transpose_into(g, bt, ag_in):
                """PE-transpose B-tile -> A-layout rows of ag_in (this rank's region)."""
                NB, MR = GEO[g]['NB'], GEO[g]['MR']
                tp = ps_tr.tile([128, MR], FR, name=f"tp_{g}", tag="tp")
                for j in range(NB):
                    nc.tensor.transpose(tp[:, j * 128:(j + 1) * 128],
                                        bt[:, j * 128:(j + 1) * 128], ident[:])
                stage = st.tile([128, MR], FR, name=f"tps_{g}", tag=f"tps_{g}")
                nc.vector.tensor_copy(stage[:], tp[:])
                nc.sync.dma_start(
                    ag_in[0:MR, :].rearrange("(j m) f -> m j f", m=128),
                    stage[:].rearrange("m (j f) -> m j f", f=BF))

            ag_count = [0]

            def allgather(g, bt):
                """Transpose B-tile `bt` to A-layout and AllGather it.
                Returns gathered [M, BF] dram tensor (rank blocks contiguous)."""
                i = ag_count[0]
                ag_count[0] += 1
                MR, M = GEO[g]['MR'], GEO[g]['M']
                ag_in = dram.tile([MR, BF], FR, name=f"agi{i}")
                ag_out, _ = tc.tile([M, BF], FR, space="DRAM",
                                    addr_space="Shared", name=f"ago{i}")
                transpose_into(g, bt, ag_in)
                nc.gpsimd.collective_compute(
                    "AllGather", mybir.AluOpType.bypass,
                    replica_groups=[list(range(NC))],
                    ins=[ag_in[:].opt()], outs=[ag_out[:].opt()],
                )
                return ag_out

            def smalls(g, idxs, rhss, act, out_name, act_bias=None):
                """psum = sum_i W[idxs[i]].T @ rhss[i]; out = act(psum [+bias])."""
                MR = GEO[g]['MR']
                psum = ps_sm.tile([BF, MR], FP, name=f"sm_{g}", tag="sm")
                n = len(idxs)
                for i, (ix, rh) in enumerate(zip(idxs, rhss)):
                    nc.tensor.matmul(psum[:], wt[g][:, ix, :], rh[:],
                                     start=(i == 0), stop=(i == n - 1))
                out = st.tile([BF, MR], FR, name=f"{out_name}_{g}", tag=f"{out_name}_{g}")
                if act_bias is not None:
                    nc.scalar.activation(out[:], psum[:], act, bias=act_bias)
                else:
                    nc.scalar.activation(out[:], psum[:], act)
                return out

            # ---- setup: Ei = L@X, Fi = L@Ei ----
            eiB, fiB, g_ei = {}, {}, {}
            for g in ('e', 'n'):
                eiB[g] = hop_to_sbuf(g, din[f'XA_{g}'], "Ei", pool=res)
                g_ei[g] = allgather(g, eiB[g])
            for g in ('e', 'n'):
                fiB[g] = hop_to_sbuf(g, g_ei[g], "Fi", pool=res)

            # ---- step 0 (hx = 0) ----
            hB, out_tiles = {}, {}
            for g in ('e', 'n'):
                stat = [xb[g], eiB[g], fiB[g]]
                r0 = smalls(g, [W_RS_X, W_RS_EI, W_RS_FI], stat, AFT.Sigmoid, "r")
                u0 = smalls(g, [W_US_X, W_US_EI, W_US_FI], stat, AFT.Sigmoid, "u")
                c0 = smalls(g, [W_CS_X, W_CS_EI, W_CS_FI], stat, AFT.Tanh, "c")
                MR = GEO[g]['MR']
                tmp = st.tile([BF, MR], FR, name=f"tmp_{g}", tag=f"tmp_{g}")
                nc.vector.tensor_mul(tmp[:], u0[:], c0[:])
                hy = st.tile([BF, MR], FR, name=f"hy_{g}", tag=f"hy_{g}")
                nc.vector.tensor_sub(hy[:], c0[:], tmp[:])
                hB[g] = hy
                y0 = smalls(g, [W_Y], [hy], AFT.Sigmoid, "y", act_bias=bias[g][:, 0:1])
                nc.sync.dma_start(dout[g].ap()[0], y0[:])

            g_hy = {}
            if T > 1:
                for g in ('e', 'n'):
                    g_hy[g] = allgather(g, hB[g])

            # ---- steps 1..T-1 (edge emitted first so node AG/compute
            # overlaps the edge AllGather windows) ----
            for t in range(1, T):
                yhB, g_yh = {}, {}
                for g in ('e', 'n'):
                    yhB[g] = hop_to_sbuf(g, g_hy[g], "Yh")
                    g_yh[g] = allgather(g, yhB[g])

                zhB, rhB, g_rh = {}, {}, {}
                for g in ('e', 'n'):
                    zhB[g] = hop_to_sbuf(g, g_yh[g], "Zh")
                    stat = [xb[g], eiB[g], fiB[g], hB[g], yhB[g], zhB[g]]
                    r = smalls(g, [W_RS_X, W_RS_EI, W_RS_FI, W_R_HX, W_R_YH, W_R_ZH],
                               stat, AFT.Sigmoid, "r")
                    u = smalls(g, [W_US_X, W_US_EI, W_US_FI, W_U_HX, W_U_YH, W_U_ZH],
                               stat, AFT.Sigmoid, "u")
                    MR = GEO[g]['MR']
                    rh = st.tile([BF, MR], FR, name=f"rh_{g}", tag=f"rh_{g}")
                    nc.vector.tensor_mul(rh[:], r[:], hB[g][:])
                    rhB[g] = rh
                    out_tiles[f'u_{g}'] = u
                    g_rh[g] = allgather(g, rh)

                yrB, g_yr = {}, {}
                for g in ('e', 'n'):
                    yrB[g] = hop_to_sbuf(g, g_rh[g], "Yr")
                    g_yr[g] = allgather(g, yrB[g])

                zrB = {}
                for g in ('e', 'n'):
                    zrB[g] = hop_to_sbuf(g, g_yr[g], "Zr")
                    c = smalls(g, [W_CS_X, W_CS_EI, W_CS_FI, W_C_RH, W_C_YR, W_C_ZR],
                               [xb[g], eiB[g], fiB[g], rhB[g], yrB[g], zrB[g]],
                               AFT.Tanh, "c")
                    u = out_tiles[f'u_{g}']
                    MR = GEO[g]['MR']
                    tmp = st.tile([BF, MR], FR, name=f"tmp_{g}", tag=f"tmp_{g}")
                    nc.vector.tensor_sub(tmp[:], hB[g][:], c[:])
                    tmp2 = st.tile([BF, MR], FR, name=f"tmp2_{g}", tag=f"tmp2_{g}")
                    nc.vector.tensor_mul(tmp2[:], u[:], tmp[:])
                    hy = st.tile([BF, MR], FR, name=f"hy_{g}", tag=f"hy_{g}")
                    nc.vector.tensor_add(hy[:], c[:], tmp2[:])
                    hB[g] = hy
                    y = smalls(g, [W_Y], [hy], AFT.Sigmoid, "y",
                               act_bias=bias[g][:, 0:1])
                    nc.sync.dma_start(dout[g].ap()[t], y[:])
                    if t < T - 1:
                        g_hy[g] = allgather(g, hy)

    nc.compile()
    return nc


_CACHE = {}


def _get_nc(T):
    if T not in _CACHE:
        _CACHE[T] = _build(T)
    return _CACHE[T]


def _in_maps(ins):
    return _prepare(**{k: ins[k] for k in (
        'inputs_node', 'inputs_edge', 'L_node', 'L_edge', 'Wg_node', 'Wg_edge',
        'Wu_node', 'Wu_edge', 'W_node', 'b_node', 'W_edge', 'b_edge')})


def _prepare(inputs_node, inputs_edge, L_node, L_edge, Wg_node, Wg_edge,
             Wu_node, Wu_edge, W_node, b_node, W_edge, b_edge):
    inputs_node = np.asarray(inputs_node, np.float32)
    inputs_edge = np.asarray(inputs_edge, np.float32)
    L_node = np.asarray(L_node, np.float32)
    L_edge = np.asarray(L_edge, np.float32)

    W_all = {
        'n': _weights_for_graph(np.asarray(Wg_node, np.float32),
                                np.asarray(Wu_node, np.float32),
                                np.asarray(W_node, np.float32)),
        'e': _weights_for_graph(np.asarray(Wg_edge, np.float32),
                                np.asarray(Wu_edge, np.float32),
                                np.asarray(W_edge, np.float32)),
    }
    XA = {'n': _a_layout(inputs_node), 'e': _a_layout(inputs_edge)}
    L_ = {'n': L_node, 'e': L_edge}
    X_ = {'n': inputs_node, 'e': inputs_edge}
    bias_ = {'n': np.asarray(b_node, np.float32), 'e': np.asarray(b_edge, np.float32)}
    ident = np.eye(128, dtype=np.float32)

    in_maps = []
    for r in range(NC):
        m = {'ident': ident}
        for g in ('e', 'n'):
            MR = GEO[g]['MR']
            m[f'LT_{g}'] = np.ascontiguousarray(L_[g][r * MR:(r + 1) * MR, :].T)
            m[f'XA_{g}'] = XA[g]
            m[f'XB_{g}'] = _b_layout_shard(X_[g], r, MR)
            m[f'W_{g}'] = W_all[g]
            m[f'bias_{g}'] = np.tile(bias_[g], B)[:, None]
        in_maps.append(m)
    return in_maps


def kernel(inputs_node, inputs_edge, L_node, L_edge, Wg_node, Wg_edge,
           Wu_node, Wu_edge, W_node, b_node, W_edge, b_edge, seq_target):
    T = int(seq_target)
    in_maps = _prepare(inputs_node, inputs_edge, L_node, L_edge, Wg_node,
                       Wg_edge, Wu_node, Wu_edge, W_node, b_node, W_edge, b_edge)
    nc = _get_nc(T)
    res = bass_utils.run_bass_kernel_spmd(nc, in_maps, core_ids=list(range(NC)))

    outs = {}
    for g, Mfull in (('n', GEO['n']['M']), ('e', GEO['e']['M'])):
        MR = GEO[g]['MR']
        full = np.empty((T, B, Mfull, F), np.float32)
        for r in range(NC):
            y = res.results[r][f'out_{g}']          # (T, BF, MR)
            y = y.reshape(T, B, F, MR).transpose(0, 1, 3, 2)
            full[:, :, r * MR:(r + 1) * MR, :] = y
        outs[g] = full
    return outs['n'], outs['e']


def timed_run(ins):
    """Re-run with NTFF tracing; returns max-core exec time in ns (or None)."""
    T = int(ins['seq_target'])
    nc = _get_nc(T)
    in_maps = _in_maps(ins)
    res = bass_utils.run_bass_kernel_spmd(nc, in_maps, core_ids=list(range(NC)),
                                          trace=True)
    return res.exec_time_ns


if __name__ == "__main__":
    # smoke test with random inputs matching spec shapes
    rng = np.random.default_rng(0)
    ins = dict(
        inputs_node=rng.standard_normal((B, 1024, F), dtype=np.float32),
        inputs_edge=rng.standard_normal((B, 4096, F), dtype=np.float32),
        L_node=(rng.standard_normal((1024, 1024)) / 32).astype(np.float32),
        L_edge=(rng.standard_normal((4096, 4096)) / 64).astype(np.float32),
        Wg_node=(rng.standard_normal((3, 64, 64)) / 8).astype(np.float32),
        Wg_edge=(rng.standard_normal((3, 64, 64)) / 8).astype(np.float32),
        Wu_node=(rng.standard_normal((3, 64, 32)) / 8).astype(np.float32),
        Wu_edge=(rng.standard_normal((3, 64, 32)) / 8).astype(np.float32),
        W_node=(rng.standard_normal((32, 32)) / 5.6).astype(np.float32),
        b_node=np.zeros(32, np.float32),
        W_edge=(rng.standard_normal((32, 32)) / 5.6).astype(np.float32),
        b_edge=np.zeros(32, np.float32),
        seq_target=6,
    )
    on, oe = kernel(**ins)
    print("node out", on.shape, "edge out", oe.shape)


# revision 16
# speedup vs baseline: 1.3431x; 1.3374x over previous
"""Trainium2 Bass kernel for nn_Decoder_GRU (Chebyshev graph-conv GRU decoder).

Strategy (8 NeuronCores, SPMD):
- Row-shard both Laplacians: edge 512 rows/core (L^T shard SBUF-resident, 8MB),
  node 128 rows/core.
- Algebra: feature transforms commute with graph propagation, so only the
  F-wide hidden state is propagated (Yh = L@hx, Zh = L@Yh); the 2F-wide concat
  never materializes. Input-dependent terms (Ei = L@X, Fi = L@Ei) are computed
  once in a setup phase and folded into each step's gate matmuls.
- Each Cheb hop contracts over the full graph dim, so the hop input is
  AllGathered across cores (fused edge+node buffers, 4 gathers/step).
- Layouts: hop outputs land "B-layout" [b*F+f, mr] (batch folded on the
  partition axis); block-diagonal weights make all feature matmuls single
  [128,128,mr] matmuls. PE-transposes produce the "A-layout" [m, b*F+f]
  shards the next hop consumes as its stationary operand.
- All matmuls run as float32r (full-rate fp32 path), typed end-to-end.

kernel(**inputs) takes the FULL inputs and returns (out_node, out_edge)
matching reference.reference().
"""
import sys
if '/opt/trn_rl_repo' not in sys.path:
    sys.path.insert(0, '/opt/trn_rl_repo')

import numpy as np
import ml_dtypes
import concourse.bacc as bacc
import concourse.mybir as mybir
import concourse.tile as tile
from concourse import bass_utils

NC = 8          # cores
B = 4           # batch
F = 32          # features
BF = B * F      # 128
FP = mybir.dt.float32
FR = mybir.dt.float32r
BT = mybir.dt.bfloat16
AFT = mybir.ActivationFunctionType

# graph geometry: (M, rows-per-core, k-chunks, A-blocks-per-shard)
GEO = {
    'e': dict(M=4096, MR=512, KC=32, NB=4),
    'n': dict(M=1024, MR=128, KC=8, NB=1),
}
AG_ROWS = GEO['e']['MR'] + GEO['n']['MR']   # 640 rows per rank in gather bufs
NODE_OFF = GEO['e']['MR']                   # node rows start at 512

# weight-tile indices in W_all [128, 19, 128]
W_RS_X, W_RS_EI, W_RS_FI = 0, 1, 2          # r statics
W_US_X, W_US_EI, W_US_FI = 3, 4, 5          # u statics
W_CS_X, W_CS_EI, W_CS_FI = 6, 7, 8          # c statics
W_R_HX, W_R_YH, W_R_ZH = 9, 10, 11          # r dynamics
W_U_HX, W_U_YH, W_U_ZH = 12, 13, 14         # u dynamics
W_C_RH, W_C_YR, W_C_ZR = 15, 16, 17         # c dynamics
W_Y = 18                                    # output proj


def _blockdiag(w):
    """w: (F, G) -> block-diag over B batches: (B*F, B*G)."""
    Fi, G = w.shape
    out = np.zeros((B * Fi, B * G), np.float32)
    for b in range(B):
        out[b * Fi:(b + 1) * Fi, b * G:(b + 1) * G] = w
    return out


def _weights_for_graph(Wg, Wu, Wy):
    """Build W_all (128, 19, 128) from Wg (3,2F,2F), Wu (3,2F,F), Wy (F,F)."""
    Wg_sum = Wg[0] - Wg[2]
    Wu_sum = Wu[0] - Wu[2]
    g_t = [Wg_sum[:F], Wg[1][:F], 2.0 * Wg[2][:F]]      # (F, 2F) tops
    g_b = [Wg_sum[F:], Wg[1][F:], 2.0 * Wg[2][F:]]      # (F, 2F) bottoms
    u_t = [Wu_sum[:F], Wu[1][:F], 2.0 * Wu[2][:F]]      # (F, F) tops
    u_b = [Wu_sum[F:], Wu[1][F:], 2.0 * Wu[2][F:]]      # (F, F) bottoms
    slots = [None] * 19
    for k in range(3):
        slots[W_RS_X + k] = g_t[k][:, :F]
        slots[W_US_X + k] = g_t[k][:, F:]
        slots[W_CS_X + k] = u_t[k]
        slots[W_R_HX + k] = g_b[k][:, :F]
        slots[W_U_HX + k] = g_b[k][:, F:]
        slots[W_C_RH + k] = u_b[k]
    slots[W_Y] = Wy
    return np.stack([_blockdiag(np.asarray(s, np.float32)) for s in slots], 1)


def _a_layout(x):
    """(B, M, F) -> A-layout (M, B*F)."""
    return np.ascontiguousarray(np.transpose(x, (1, 0, 2)).reshape(x.shape[1], BF))


def _b_layout_shard(x, r, mr):
    """(B, M, F) -> B-layout shard (B*F, mr) for core r."""
    sl = x[:, r * mr:(r + 1) * mr, :]           # (B, mr, F)
    return np.ascontiguousarray(np.transpose(sl, (0, 2, 1)).reshape(BF, mr))


def _build(T):
    nc = bacc.Bacc("TRN2", target_bir_lowering=False, debug=False, num_devices=NC)

    # ---- DRAM I/O ----
    din = {}
    for g in ('e', 'n'):
        M, MR = GEO[g]['M'], GEO[g]['MR']
        din[f'LT_{g}'] = nc.dram_tensor(f"LT_{g}", [M, MR], BT, kind="ExternalInput")
        din[f'XA_{g}'] = nc.dram_tensor(f"XA_{g}", [M, BF], BT, kind="ExternalInput")
        din[f'XB_{g}'] = nc.dram_tensor(f"XB_{g}", [BF, MR], FR, kind="ExternalInput")
        din[f'W_{g}'] = nc.dram_tensor(f"W_{g}", [BF, 19, BF], FR, kind="ExternalInput")
        din[f'bias_{g}'] = nc.dram_tensor(f"bias_{g}", [BF, 1], FP, kind="ExternalInput")
    din['ident'] = nc.dram_tensor("ident", [128, 128], FR, kind="ExternalInput")
    dout = {
        'e': nc.dram_tensor("out_e", [T, BF, GEO['e']['MR']], FR, kind="ExternalOutput"),
        'n': nc.dram_tensor("out_n", [T, BF, GEO['n']['MR']], FR, kind="ExternalOutput"),
    }

    with tile.TileContext(nc) as tc:
        with (
            tc.tile_pool(name="res", bufs=1) as res,            # resident tiles
            tc.tile_pool(name="st", bufs=2) as st,              # per-step state tiles
            tc.tile_pool(name="lhsT_e", bufs=16) as lhsT_e,     # hop stationary stream
            tc.tile_pool(name="lhsT_n", bufs=4) as lhsT_n,
            tc.tile_pool(name="ps_hop", bufs=2, space="PSUM") as ps_hop,
            tc.tile_pool(name="ps_tr", bufs=2, space="PSUM") as ps_tr,
            tc.tile_pool(name="ps_sm", bufs=2, space="PSUM") as ps_sm,
            tc.tile_pool(name="dram", bufs=1, space="DRAM") as dram,
        ):
            # ---- resident loads ----
            lt = {}
            for g in ('e', 'n'):
                M, MR, KC = GEO[g]['M'], GEO[g]['MR'], GEO[g]['KC']
                tiles = []
                ngrp = KC // 8
                for gi in range(ngrp):
                    t = res.tile([128, 8, MR], BT, name=f"lt_{g}{gi}")
                    nc.sync.dma_start(
                        t[:], din[f'LT_{g}'].ap()[gi * 1024:(gi + 1) * 1024, :]
                        .rearrange("(c p) m -> p c m", p=128))
                    tiles.append(t)
                lt[g] = tiles

            xb, wt, bias = {}, {}, {}
            for g in ('e', 'n'):
                MR = GEO[g]['MR']
                xb[g] = res.tile([BF, MR], FR, name=f"xb_{g}")
                nc.sync.dma_start(xb[g][:], din[f'XB_{g}'][:])
                wt[g] = res.tile([BF, 19, BF], FR, name=f"wt_{g}")
                nc.sync.dma_start(wt[g][:], din[f'W_{g}'][:])
                bias[g] = res.tile([BF, 1], FP, name=f"bias_{g}")
                nc.sync.dma_start(bias[g][:], din[f'bias_{g}'][:])
            ident = res.tile([128, 128], FR, name="ident")
            nc.sync.dma_start(ident[:], din['ident'][:])

            def ltc(g, k):
                return lt[g][k // 8][:, k % 8, :]

            # ---- helpers ----
            def hop(g, src_dram, tag):
                """Yout.T[bf, mr] = sum_m src[m, bf] * LT[m, mr].
                src is a contiguous [M, BF] dram tensor."""
                M, MR, KC, NB = (GEO[g][k] for k in ('M', 'MR', 'KC', 'NB'))
                psum = ps_hop.tile([BF, MR], FP, name=f"hops_{g}", tag="hop")
                chunks = []
                if g == 'e':
                    for rr in range(NC):
                        tl = lhsT_e.tile([128, NB, BF], BT,
                                         name=f"lh_{tag}{rr}", tag="lhsT_e")
                        base = rr * MR
                        nc.sync.dma_start(
                            tl[:], src_dram[base:base + MR, :]
                            .rearrange("(j p) f -> p j f", p=128))
                        for j in range(NB):
                            chunks.append(tl[:, j, :])
                else:
                    for half in range(2):
                        tl = lhsT_n.tile([128, 4, BF], BT,
                                         name=f"lh_{tag}h{half}", tag="lhsT_n")
                        rr0 = half * 4
                        src = (src_dram[rr0 * 128:(rr0 + 4) * 128, :]
                               .rearrange("(c p) f -> p c f", p=128))
                        nc.sync.dma_start(tl[:], src)
                        for j in range(4):
                            chunks.append(tl[:, j, :])
                for k in range(KC):
                    nc.tensor.matmul(psum[:], chunks[k], ltc(g, k),
                                     start=(k == 0), stop=(k == KC - 1))
                return psum

            def hop_to_sbuf(g, src_dram, tag, pool=None):
                psum = hop(g, src_dram, tag)
                out = (pool or st).tile([BF, GEO[g]['MR']], FR,
                                        name=f"{tag}_{g}", tag=f"{tag}_{g}")
                nc.vector.tensor_copy(out[:], psum[:])
                return out

            def transpose_into(g, bt, ag_in):
                """PE-transpose B-tile -> A-layout rows of ag_in (this rank's region)."""
                NB, MR = GEO[g]['NB'], GEO[g]['MR']
                tp = ps_tr.tile([128, MR], FR, name=f"tp_{g}", tag="tp")
                for j in range(NB):
                    nc.tensor.transpose(tp[:, j * 128:(j + 1) * 128],
                                        bt[:, j * 128:(j + 1) * 128], ident[:])
                stage = st.tile([128, MR], BT, name=f"tps_{g}", tag=f"tps_{g}")
                nc.vector.tensor_copy(stage[:], tp[:])
                nc.sync.dma_start(
                    ag_in[0:MR, :].rearrange("(j m) f -> m j f", m=128),
                    stage[:].rearrange("m (j f) -> m j f", f=BF))

            ag_count = [0]

            def allgather(g, bt):
                """Transpose B-tile `bt` to A-layout and AllGather it.
                Returns gathered [M, BF] dram tensor (rank blocks contiguous)."""
                i = ag_count[0]
                ag_count[0] += 1
                MR, M = GEO[g]['MR'], GEO[g]['M']
                ag_in = dram.tile([MR, BF], BT, name=f"agi{i}")
                ag_out = dram.tile([M, BF], BT, name=f"ago{i}",
                                    addr_space="Shared")
                transpose_into(g, bt, ag_in)
                nc.gpsimd.collective_compute(
                    "AllGather", mybir.AluOpType.bypass,
                    replica_groups=[list(range(NC))],
                    ins=[ag_in[:].opt()], outs=[ag_out[:].opt()],
                )
                return ag_out

            def smalls(g, idxs, rhss, act, out_name, act_bias=None):
                """psum = sum_i W[idxs[i]].T @ rhss[i]; out = act(psum [+bias])."""
                MR = GEO[g]['MR']
                psum = ps_sm.tile([BF, MR], FP, name=f"sm_{g}", tag="sm")
                n = len(idxs)
                for i, (ix, rh) in enumerate(zip(idxs, rhss)):
                    nc.tensor.matmul(psum[:], wt[g][:, ix, :], rh[:],
                                     start=(i == 0), stop=(i == n - 1))
                out = st.tile([BF, MR], FR, name=f"{out_name}_{g}", tag=f"{out_name}_{g}")
                if act_bias is not None:
                    nc.scalar.activation(out[:], psum[:], act, bias=act_bias)
                else:
                    nc.scalar.activation(out[:], psum[:], act)
                return out

            # ---- setup: Ei = L@X, Fi = L@Ei ----
            eiB, fiB, g_ei = {}, {}, {}
            for g in ('e', 'n'):
                eiB[g] = hop_to_sbuf(g, din[f'XA_{g}'], "Ei", pool=res)
                g_ei[g] = allgather(g, eiB[g])
            for g in ('e', 'n'):
                fiB[g] = hop_to_sbuf(g, g_ei[g], "Fi", pool=res)

            # ---- step 0 (hx = 0) ----
            hB, out_tiles = {}, {}
            for g in ('e', 'n'):
                stat = [xb[g], eiB[g], fiB[g]]
                r0 = smalls(g, [W_RS_X, W_RS_EI, W_RS_FI], stat, AFT.Sigmoid, "r")
                u0 = smalls(g, [W_US_X, W_US_EI, W_US_FI], stat, AFT.Sigmoid, "u")
                c0 = smalls(g, [W_CS_X, W_CS_EI, W_CS_FI], stat, AFT.Tanh, "c")
                MR = GEO[g]['MR']
                tmp = st.tile([BF, MR], FR, name=f"tmp_{g}", tag=f"tmp_{g}")
                nc.vector.tensor_mul(tmp[:], u0[:], c0[:])
                hy = st.tile([BF, MR], FR, name=f"hy_{g}", tag=f"hy_{g}")
                nc.vector.tensor_sub(hy[:], c0[:], tmp[:])
                hB[g] = hy
                y0 = smalls(g, [W_Y], [hy], AFT.Sigmoid, "y", act_bias=bias[g][:, 0:1])
                nc.sync.dma_start(dout[g].ap()[0], y0[:])

            g_hy = {}
            if T > 1:
                for g in ('e', 'n'):
                    g_hy[g] = allgather(g, hB[g])

            # ---- steps 1..T-1 (edge emitted first so node AG/compute
            # overlaps the edge AllGather windows) ----
            for t in range(1, T):
                yhB, g_yh = {}, {}
                for g in ('e', 'n'):
                    yhB[g] = hop_to_sbuf(g, g_hy[g], "Yh")
                    g_yh[g] = allgather(g, yhB[g])

                zhB, rhB, g_rh = {}, {}, {}
                for g in ('e', 'n'):
                    zhB[g] = hop_to_sbuf(g, g_yh[g], "Zh")
                    stat = [xb[g], eiB[g], fiB[g], hB[g], yhB[g], zhB[g]]
                    r = smalls(g, [W_RS_X, W_RS_EI, W_RS_FI, W_R_HX, W_R_YH, W_R_ZH],
                               stat, AFT.Sigmoid, "r")
                    u = smalls(g, [W_US_X, W_US_EI, W_US_FI, W_U_HX, W_U_YH, W_U_ZH],
                               stat, AFT.Sigmoid, "u")
                    MR = GEO[g]['MR']
                    rh = st.tile([BF, MR], FR, name=f"rh_{g}", tag=f"rh_{g}")
                    nc.vector.tensor_mul(rh[:], r[:], hB[g][:])
                    rhB[g] = rh
                    out_tiles[f'u_{g}'] = u
                    g_rh[g] = allgather(g, rh)

                yrB, g_yr = {}, {}
                for g in ('e', 'n'):
                    yrB[g] = hop_to_sbuf(g, g_rh[g], "Yr")
                    g_yr[g] = allgather(g, yrB[g])

                zrB = {}
                for g in ('e', 'n'):
                    zrB[g] = hop_to_sbuf(g, g_yr[g], "Zr")
                    c = smalls(g, [W_CS_X, W_CS_EI, W_CS_FI, W_C_RH, W_C_YR, W_C_ZR],
                               [xb[g], eiB[g], fiB[g], rhB[g], yrB[g], zrB[g]],
                               AFT.Tanh, "c")
                    u = out_tiles[f'u_{g}']
                    MR = GEO[g]['MR']
                    tmp = st.tile([BF, MR], FR, name=f"tmp_{g}", tag=f"tmp_{g}")
                    nc.vector.tensor_sub(tmp[:], hB[g][:], c[:])
                    tmp2 = st.tile([BF, MR], FR, name=f"tmp2_{g}", tag=f"tmp2_{g}")
                    nc.vector.tensor_mul(tmp2[:], u[:], tmp[:])
                    hy = st.tile([BF, MR], FR, name=f"hy_{g}", tag=f"hy_{g}")
                    nc.vector.tensor_add(hy[:], c[:], tmp2[:])
                    hB[g] = hy
                    y = smalls(g, [W_Y], [hy], AFT.Sigmoid, "y",
                               act_bias=bias[g][:, 0:1])
                    nc.sync.dma_start(dout[g].ap()[t], y[:])
                    if t < T - 1:
                        g_hy[g] = allgather(g, hy)

    nc.compile()
    return nc


_CACHE = {}


def _get_nc(T):
    if T not in _CACHE:
        _CACHE[T] = _build(T)
    return _CACHE[T]


def _in_maps(ins):
    return _prepare(**{k: ins[k] for k in (
        'inputs_node', 'inputs_edge', 'L_node', 'L_edge', 'Wg_node', 'Wg_edge',
        'Wu_node', 'Wu_edge', 'W_node', 'b_node', 'W_edge', 'b_edge')})


def _prepare(inputs_node, inputs_edge, L_node, L_edge, Wg_node, Wg_edge,
             Wu_node, Wu_edge, W_node, b_node, W_edge, b_edge):
    inputs_node = np.asarray(inputs_node, np.float32)
    inputs_edge = np.asarray(inputs_edge, np.float32)
    L_node = np.asarray(L_node, np.float32)
    L_edge = np.asarray(L_edge, np.float32)

    W_all = {
        'n': _weights_for_graph(np.asarray(Wg_node, np.float32),
                                np.asarray(Wu_node, np.float32),
                                np.asarray(W_node, np.float32)),
        'e': _weights_for_graph(np.asarray(Wg_edge, np.float32),
                                np.asarray(Wu_edge, np.float32),
                                np.asarray(W_edge, np.float32)),
    }
    XA = {'n': _a_layout(inputs_node), 'e': _a_layout(inputs_edge)}
    L_ = {'n': L_node, 'e': L_edge}
    X_ = {'n': inputs_node, 'e': inputs_edge}
    bias_ = {'n': np.asarray(b_node, np.float32), 'e': np.asarray(b_edge, np.float32)}
    ident = np.eye(128, dtype=np.float32)

    in_maps = []
    for r in range(NC):
        m = {'ident': ident}
        for g in ('e', 'n'):
            MR = GEO[g]['MR']
            m[f'LT_{g}'] = np.ascontiguousarray(L_[g][r * MR:(r + 1) * MR, :].T).astype(ml_dtypes.bfloat16)
            m[f'XA_{g}'] = XA[g].astype(ml_dtypes.bfloat16)
            m[f'XB_{g}'] = _b_layout_shard(X_[g], r, MR)
            m[f'W_{g}'] = W_all[g]
            m[f'bias_{g}'] = np.tile(bias_[g], B)[:, None]
        in_maps.append(m)
    return in_maps


def kernel(inputs_node, inputs_edge, L_node, L_edge, Wg_node, Wg_edge,
           Wu_node, Wu_edge, W_node, b_node, W_edge, b_edge, seq_target):
    T = int(seq_target)
    in_maps = _prepare(inputs_node, inputs_edge, L_node, L_edge, Wg_node,
                       Wg_edge, Wu_node, Wu_edge, W_node, b_node, W_edge, b_edge)
    nc = _get_nc(T)
    res = bass_utils.run_bass_kernel_spmd(nc, in_maps, core_ids=list(range(NC)))

    outs = {}
    for g, Mfull in (('n', GEO['n']['M']), ('e', GEO['e']['M'])):
        MR = GEO[g]['MR']
        full = np.empty((T, B, Mfull, F), np.float32)
        for r in range(NC):
            y = res.results[r][f'out_{g}']          # (T, BF, MR)
            y = y.reshape(T, B, F, MR).transpose(0, 1, 3, 2)
            full[:, :, r * MR:(r + 1) * MR, :] = y
        outs[g] = full
    return outs['n'], outs['e']


def timed_run(ins):
    """Re-run with NTFF tracing; returns max-core exec time in ns (or None)."""
    T = int(ins['seq_target'])
    nc = _get_nc(T)
    in_maps = _in_maps(ins)
    res = bass_utils.run_bass_kernel_spmd(nc, in_maps, core_ids=list(range(NC)),
                                          trace=True)
    return res.exec_time_ns


if __name__ == "__main__":
    # smoke test with random inputs matching spec shapes
    rng = np.random.default_rng(0)
    ins = dict(
        inputs_node=rng.standard_normal((B, 1024, F), dtype=np.float32),
        inputs_edge=rng.standard_normal((B, 4096, F), dtype=np.float32),
        L_node=(rng.standard_normal((1024, 1024)) / 32).astype(np.float32),
        L_edge=(rng.standard_normal((4096, 4096)) / 64).astype(np.float32),
        Wg_node=(rng.standard_normal((3, 64, 64)) / 8).astype(np.float32),
        Wg_edge=(rng.standard_normal((3, 64, 64)) / 8).astype(np.float32),
        Wu_node=(rng.standard_normal((3, 64, 32)) / 8).astype(np.float32),
        Wu_edge=(rng.standard_normal((3, 64, 32)) / 8).astype(np.float32),
        W_node=(rng.standard_normal((32, 32)) / 5.6).astype(np.float32),
        b_node=np.zeros(32, np.float32),
        W_edge=(rng.standard_normal((32, 32)) / 5.6).astype(np.float32),
        b_edge=np.zeros(32, np.float32),
        seq_target=6,
    )
    on, oe = kernel(**ins)
    print("node out", on.shape, "edge out", oe.shape)
